# revision 29
# baseline (speedup 1.0000x reference)
"""Trainium2 Bass kernel for nn_CameraEstimator.

Computes, for each batch item b:
    camera[b] = einsum('chw,c->hw', x[b], W)          (C=256 contraction)
    out[b]    = nearest-rotation(camera[b])           (SVD u@vh + det reflection fix)

Pipeline per 128-row tile (32 tiles/core):
    SWDGE DMA with inline fp32->fp16 cast -> PE transpose (18x [128,128]) ->
    DVE/ACT copy PSUM->SBUF -> PE matmul vs masked-W (accumulate C) -> cam.
The SO(3) projection (scaled Newton polar + closed-form smallest-eigenvalue
reflection fix) runs on DVE/ACT in 3 chunks interleaved with the tile loop.
To shorten the serial dependency chain of the last chunk, the reflection's
P-factor is computed from the 2nd Newton iterate (numerically equivalent:
rel err 7.8e-4 either way) so the eigen/projector branch overlaps Newton
iterations 3-4.  All ACT transcendentals are Sqrt (one table set);
sin(acos(r)/3+pi/6) is a pair of factored quartics P(r)+sqrt(1-r)*Q(r).

Sharding: batch dim split evenly across 8 NeuronCores (data parallel), W
replicated.
"""

import numpy as np

import concourse.bacc as bacc
import concourse.bass as bass
import concourse.mybir as mybir
from concourse.bass_types import AP
from concourse.tile import TileContext
from concourse import bass_utils

F32 = mybir.dt.float32
F16 = mybir.dt.float16
ALU = mybir.AluOpType
ACT = mybir.ActivationFunctionType

B_FULL = 32768
C = 256
E = 9
N_CORES = 8
P = 128
B_LOCAL = B_FULL // N_CORES          # 4096
TPC = B_LOCAL // P                   # 32 matrices per partition
NCH = (C * E) // P                   # 18 chunks of 128 per tile

CHUNKS = [(0, 16), (16, 26), (26, 32)]
DVE_COPY_TILES = {1, 4, 7, 10, 13, 16, 19, 22, 25}  # PSUM copies on DVE
SPLIT_COPY_TILES = 28                 # tiles >= this split copies DVE/ACT
# x DMA grouping: small transfers first (fast pipeline start), then 4-tile
# groups (fewer transfers -> no completion-sem reuse throttling)
DMA_GROUPS = [1] * TPC

# sin(acos(r)/3 + pi/6) ~= c4P*(r^2+pb1*r+pb0)(r^2+pb3*r+pb2)
#                        + sqrt(1-r)*c4Q*(...)
C4P = -0.00012669774781398082
PB = [(-10.373862952604547, -41.02339770448056),
      (-6.311076547771094, 83.61692215433715)]
C4Q = 1.3807082105156778e-05
QB = [(-46.779313658342474, 452.0278334014067),
      (2.6583141961744774, 69.12475662024718)]


def v(base: AP, off: int, *dims) -> AP:
    """Free-dim view of an SBUF tile AP: keep partition dim, set free dims."""
    return AP(base.tensor, base.offset + off,
              [list(base.ap[0])] + [[s, c] for (s, c) in dims])


def make_wm(W: np.ndarray) -> np.ndarray:
    """Masked-W moving operand, partition-major: wm[k, j*9+m] = fp16(W[c])
    where c=(128j+k)//9 if (128j+k)%9 == m else 0, so that
    xT16_j.T @ wm[:, 9j:9j+9] accumulates camera.  Layout [P, NCH*E] keeps
    the DMA contiguous per partition (tiny-descriptor DMAs stall the SDMA
    round-robin and starve the x stream)."""
    kidx = np.arange(C * E)
    wh = np.zeros((C * E, E), np.float32)
    wh[kidx, kidx % E] = W[kidx // E]
    wjkm = wh.astype(np.float16).reshape(NCH, P, E)
    return np.ascontiguousarray(wjkm.transpose(1, 0, 2).reshape(P, NCH * E))


def _emit(nc, tc, x_ap, wm_ap, idt_ap, y_ap):
    f32 = F32
    vec = nc.vector
    act = nc.scalar

    x_flat = x_ap.rearrange("b c h w -> b (c h w)")
    x_tiled = x_flat.rearrange("(p t) f -> p t f", p=P)
    y_flat = y_ap.rearrange("b h w -> b (h w)").rearrange("(p t) e -> p (t e)", p=P)

    NMAX = max(t1 - t0 for t0, t1 in CHUNKS)

    with tc.tile_pool(name="x16", bufs=8) as x16pool, \
         tc.tile_pool(name="tp", bufs=6, space="PSUM") as tpp, \
         tc.tile_pool(name="pcp", bufs=2, space="PSUM") as pcp, \
         tc.tile_pool(name="cam", bufs=2) as campool, \
         tc.tile_pool(name="ck", bufs=2) as ck, \
         tc.tile_pool(name="wk", bufs=1) as wp:

        idt = wp.tile([P, P], F16)
        nc.sync.dma_start(out=idt[:], in_=idt_ap)
        wm_sb = wp.tile([P, NCH * E], F16)
        nc.sync.dma_start(out=wm_sb[:], in_=wm_ap)

        _consts = {}

        def cb(val):
            if val not in _consts:
                ct = wp.tile([P, 1], f32, name=f"const{len(_consts)}")
                vec.memset(ct[:], float(val))
                _consts[val] = ct[:]
            return _consts[val]

        # ---------------- SO(3) projection for one chunk -------------------
        # mu schedule per Newton iter: "38" = |det|^(-3/8) (4 sqrt levels),
        # "12" = |det|^(-1/2) (2 sqrt levels), None = unscaled.
        MU_SCHED = ("38", "38", "12", None)

        def so3_begin(ci, n):
            nm = f"c{ci}"

            def big(tag):
                return ck.tile([P, NMAX * E], f32, tag=tag, name=f"{tag}{nm}")

            def plane(tag):
                return ck.tile([P, NMAX], f32, tag=tag, name=f"{tag}{nm}")

            S = {"ci": ci, "n": n, "last": ci == len(CHUNKS) - 1}
            for t_ in ("Ya", "Yb", "Yc", "CfN", "t1", "t2",
                       "CfR", "ra", "rb", "rc"):
                S[t_] = big(t_)
            S["D"] = ck.tile([P, NMAX * 36], f32, tag="D", name=f"D{nm}")
            S["D2"] = ck.tile([P, NMAX * 36], f32, tag="D2", name=f"D2{nm}")
            S["td"] = ck.tile([P, NMAX * 3], f32, tag="td", name=f"td{nm}")
            S["td2"] = ck.tile([P, NMAX * 3], f32, tag="td2", name=f"td2{nm}")
            for t_ in ("det", "det0", "rdet", "mi", "u1", "u2", "u3",
                       "c2", "c1", "c0", "q", "r", "p26", "pp",
                       "sq", "ha", "hb", "hp", "s3", "w1", "plv"):
                S[t_] = plane(t_)
            return S

        def matv(tile, k0, m, off=0):
            return v(tile, k0 * E + off, (E, m), (3, 3), (1, 3))

        def flatv(tile, k0, m):
            return v(tile, k0 * E, (1, m * E))

        def row0v(tile, k0, m):
            return v(tile, k0 * E, (E, m), (1, 3))

        def diagv(tile, k0, m):
            return v(tile, k0 * E, (E, m), (4, 3))

        def plv_(tile, k0, m):
            return v(tile, k0, (1, m))

        def bc9v(tile, k0, m):
            return v(tile, k0, (1, m), (0, E))

        def bc3v(tile, k0, m):
            return v(tile, k0, (1, m), (0, 3))

        def build_D(S, Y, Dst, k0, m):
            src = v(Y, k0 * E, (E, m), (3, 3), (0, 2), (1, 3))
            eng = vec.tensor_copy if S["last"] else act.copy
            for off in (0, 18):
                eng(v(Dst, k0 * 36 + off, (36, m), (6, 3), (3, 2), (1, 3)),
                    src)

        def cofactor(S, Y, out, Dst, ta, tb, k0, m):
            def db(off):
                return v(Dst, k0 * 36 + off, (36, m), (6, 3), (1, 3))
            build_D(S, Y, Dst, k0, m)
            vec.tensor_tensor(matv(ta, k0, m), db(7), db(14), ALU.mult)
            vec.tensor_tensor(matv(tb, k0, m), db(8), db(13), ALU.mult)
            vec.tensor_tensor(matv(out, k0, m), matv(ta, k0, m),
                              matv(tb, k0, m), ALU.subtract)

        def det_of(S, Y, Cof, out, tdx, k0, m):
            vec.tensor_tensor(v(tdx, k0 * 3, (3, m), (1, 3)),
                              row0v(Y, k0, m), row0v(Cof, k0, m), ALU.mult)
            vec.tensor_reduce(plv_(out, k0, m), v(tdx, k0 * 3, (3, m), (1, 3)),
                              mybir.AxisListType.X, ALU.add)

        def newton_iter(S, Y, Yn, it, mu, k0, m):
            det, rdet, mi = S["det"], S["rdet"], S["mi"]
            u1, u2 = S["u1"], S["u2"]
            cofactor(S, Y, S["CfN"], S["D"], S["t1"], S["t2"], k0, m)
            det_of(S, Y, S["CfN"], det, S["td"], k0, m)
            if it == 0:
                vec.tensor_copy(plv_(S["det0"], k0, m), plv_(det, k0, m))
            vec.reciprocal(plv_(rdet, k0, m), plv_(det, k0, m))
            if mu == "38":
                # 1/mu = n^(1/4)*n^(1/8), n = sqrt(det^2)
                vec.tensor_tensor(plv_(u1, k0, m), plv_(det, k0, m),
                                  plv_(det, k0, m), ALU.mult)
                act.activation(plv_(u1, k0, m), plv_(u1, k0, m), ACT.Sqrt,
                               bias=cb(1e-35))
                act.activation(plv_(u1, k0, m), plv_(u1, k0, m), ACT.Sqrt,
                               bias=cb(0.0))
                act.activation(plv_(u2, k0, m), plv_(u1, k0, m), ACT.Sqrt,
                               bias=cb(0.0))
                act.activation(plv_(S["u3"], k0, m), plv_(u2, k0, m), ACT.Sqrt,
                               bias=cb(0.0))
                vec.tensor_tensor(plv_(mi, k0, m), plv_(u2, k0, m),
                                  plv_(S["u3"], k0, m), ALU.mult)
            elif mu == "12":
                # 1/mu = n^(1/2)
                vec.tensor_tensor(plv_(u1, k0, m), plv_(det, k0, m),
                                  plv_(det, k0, m), ALU.mult)
                act.activation(plv_(u1, k0, m), plv_(u1, k0, m), ACT.Sqrt,
                               bias=cb(1e-35))
                act.activation(plv_(mi, k0, m), plv_(u1, k0, m), ACT.Sqrt,
                               bias=cb(0.0))
            if mu is not None:
                vec.scalar_tensor_tensor(plv_(u2, k0, m), plv_(mi, k0, m), 0.5,
                                         plv_(rdet, k0, m), ALU.mult, ALU.mult)
                vec.reciprocal(plv_(u1, k0, m), plv_(mi, k0, m))
                vec.tensor_tensor(flatv(S["t1"], k0, m), flatv(Y, k0, m),
                                  bc9v(u1, k0, m), ALU.mult)
                vec.tensor_tensor(flatv(S["t2"], k0, m), flatv(S["CfN"], k0, m),
                                  bc9v(u2, k0, m), ALU.mult)
                vec.scalar_tensor_tensor(flatv(Yn, k0, m), flatv(S["t1"], k0, m),
                                         0.5, flatv(S["t2"], k0, m),
                                         ALU.mult, ALU.add)
            else:
                vec.scalar_tensor_tensor(flatv(S["t2"], k0, m),
                                         flatv(S["CfN"], k0, m), 0.5,
                                         bc9v(rdet, k0, m), ALU.mult, ALU.mult)
                vec.scalar_tensor_tensor(flatv(Yn, k0, m), flatv(Y, k0, m),
                                         0.5, flatv(S["t2"], k0, m),
                                         ALU.mult, ALU.add)

        def so3_iter0_tile(S, cam_c, k):
            newton_iter(S, cam_c, S["Ya"], 0, MU_SCHED[0], k, 1)

        def so3_rest(S, cam_c, pre0):
            ci, n = S["ci"], S["n"]
            NE = n * E

            def mat(t_, off=0):
                return matv(t_, 0, n, off)

            def flat(t_):
                return flatv(t_, 0, n)

            def diag(t_):
                return diagv(t_, 0, n)

            def pl(t_):
                return plv_(t_, 0, n)

            def bc9(t_):
                return bc9v(t_, 0, n)

            def bc3(t_):
                return bc3v(t_, 0, n)

            Ya, Yb, Yc = S["Ya"], S["Yb"], S["Yc"]
            CfR, ra, rb, rc = S["CfR"], S["ra"], S["rb"], S["rc"]
            t1 = S["t1"]
            det0 = S["det0"]
            c2, c1, c0 = S["c2"], S["c1"], S["c0"]
            q, r, p26, pp = S["q"], S["r"], S["p26"], S["pp"]
            sq, ha, hb, hp = S["sq"], S["ha"], S["hb"], S["hp"]
            s3, w1, plv = S["s3"], S["w1"], S["plv"]

            if not pre0:
                newton_iter(S, cam_c, Ya, 0, MU_SCHED[0], 0, n)
            newton_iter(S, Ya, Yb, 1, MU_SCHED[1], 0, n)

            # ---- reflection prep from Y2 (=Yb), concurrent with iters 3-4 --
            Pm = rb
            for k in range(3):
                a = v(Yb, 3 * k, (E, n), (1, 3), (0, 3))
                b = v(cam_c, 3 * k, (E, n), (0, 3), (1, 3))
                dst = (Pm, ra, CfR)[k]
                vec.tensor_tensor(mat(dst), a, b, ALU.mult)
            vec.tensor_tensor(mat(ra), mat(ra), mat(CfR), ALU.add)
            vec.tensor_tensor(mat(Pm), mat(Pm), mat(ra), ALU.add)

            cofactor(S, Pm, CfR, S["D2"], ra, rc, 0, n)
            vec.tensor_reduce(pl(c2), diag(Pm), mybir.AxisListType.X, ALU.add)
            vec.tensor_reduce(pl(c1), diag(CfR), mybir.AxisListType.X, ALU.add)
            det_of(S, Pm, CfR, c0, S["td2"], 0, n)

            vec.tensor_scalar(pl(q), pl(c2), 1.0 / 3.0, None, ALU.mult)
            vec.scalar_tensor_tensor(pl(p26), pl(c2), 1.0 / 9.0, pl(c2),
                                     ALU.mult, ALU.mult)
            vec.scalar_tensor_tensor(pl(p26), pl(c1), -1.0 / 3.0, pl(p26),
                                     ALU.mult, ALU.add)
            vec.tensor_scalar(pl(p26), pl(p26), 0.0, None, ALU.max)
            act.activation(pl(pp), pl(p26), ACT.Sqrt, bias=cb(1e-30))
            vec.scalar_tensor_tensor(pl(r), pl(c2), 2.0 / 9.0, pl(c2),
                                     ALU.mult, ALU.mult)
            vec.tensor_tensor(pl(r), pl(r), pl(c1), ALU.subtract)
            vec.tensor_tensor(pl(r), pl(r), pl(q), ALU.mult)
            vec.tensor_tensor(pl(r), pl(r), pl(c0), ALU.add)
            vec.scalar_tensor_tensor(pl(plv), pl(p26), 2.0, pl(pp),
                                     ALU.mult, ALU.mult)
            vec.tensor_scalar(pl(plv), pl(plv), 1e-30, None, ALU.add)
            vec.reciprocal(pl(plv), pl(plv))
            vec.tensor_tensor(pl(r), pl(r), pl(plv), ALU.mult)
            vec.tensor_scalar(pl(r), pl(r), -1.0, 1.0, ALU.max, ALU.min)
            act.activation(pl(sq), pl(r), ACT.Sqrt, scale=-1.0, bias=cb(1.0))
            vec.tensor_tensor(pl(S["u3"]), pl(r), pl(r), ALU.mult)  # r^2
            r2 = S["u3"]
            vec.scalar_tensor_tensor(pl(ha), pl(r), PB[0][0], pl(r2),
                                     ALU.mult, ALU.add)
            vec.tensor_scalar(pl(ha), pl(ha), PB[0][1], None, ALU.add)
            vec.scalar_tensor_tensor(pl(hb), pl(r), PB[1][0], pl(r2),
                                     ALU.mult, ALU.add)
            vec.tensor_scalar(pl(hb), pl(hb), PB[1][1], None, ALU.add)
            vec.scalar_tensor_tensor(pl(hp), pl(ha), C4P, pl(hb),
                                     ALU.mult, ALU.mult)
            vec.scalar_tensor_tensor(pl(ha), pl(r), QB[0][0], pl(r2),
                                     ALU.mult, ALU.add)
            vec.tensor_scalar(pl(ha), pl(ha), QB[0][1], None, ALU.add)
            vec.scalar_tensor_tensor(pl(hb), pl(r), QB[1][0], pl(r2),
                                     ALU.mult, ALU.add)
            vec.tensor_scalar(pl(hb), pl(hb), QB[1][1], None, ALU.add)
            vec.scalar_tensor_tensor(pl(ha), pl(ha), C4Q, pl(hb),
                                     ALU.mult, ALU.mult)
            vec.tensor_tensor(pl(ha), pl(ha), pl(sq), ALU.mult)
            vec.tensor_tensor(pl(hp), pl(hp), pl(ha), ALU.add)
            vec.scalar_tensor_tensor(pl(s3), pl(pp), -2.0, pl(hp),
                                     ALU.mult, ALU.mult)
            vec.tensor_tensor(pl(s3), pl(s3), pl(q), ALU.add)
            # Nadj = CP + s3*P + (s3^2 - s3*c2) I; proj = Nadj/tr * mask
            vec.scalar_tensor_tensor(pl(w1), pl(c2), -1.0, pl(s3),
                                     ALU.mult, ALU.add)
            vec.tensor_tensor(pl(w1), pl(w1), pl(s3), ALU.mult)
            vec.tensor_tensor(flat(ra), flat(Pm), bc9(s3), ALU.mult)
            vec.tensor_tensor(flat(CfR), flat(CfR), flat(ra), ALU.add)
            vec.tensor_tensor(diag(CfR), diag(CfR), bc3(w1), ALU.add)
            vec.tensor_reduce(pl(plv), diag(CfR), mybir.AxisListType.X, ALU.add)
            vec.tensor_scalar(pl(plv), pl(plv), 1e-30, None, ALU.add)
            vec.reciprocal(pl(plv), pl(plv))
            # fold the det0<0 reflection mask into proj (off the tail path)
            vec.tensor_scalar(pl(ha), pl(det0), 0.0, 2.0, ALU.is_lt, ALU.mult)
            vec.tensor_tensor(pl(plv), pl(plv), pl(ha), ALU.mult)
            vec.tensor_tensor(flat(CfR), flat(CfR), bc9(plv), ALU.mult)

            # Newton iters 3-4 (emitted after prep; scheduler overlaps)
            newton_iter(S, Yb, Yc, 2, MU_SCHED[2], 0, n)
            newton_iter(S, Yc, Ya, 3, MU_SCHED[3], 0, n)
            orth = Ya

            # corr = orth @ proj (tree); R = orth - clamp(corr)
            corr = rb  # Pm dead after Nadj
            for k in range(3):
                a = v(orth, k, (E, n), (3, 3), (0, 3))
                b = v(CfR, 3 * k, (E, n), (0, 3), (1, 3))
                dst = (corr, ra, t1)[k]
                vec.tensor_tensor(mat(dst), a, b, ALU.mult)
            vec.tensor_tensor(mat(ra), mat(ra), mat(t1), ALU.add)
            vec.tensor_tensor(mat(corr), mat(corr), mat(ra), ALU.add)
            vec.tensor_scalar(flat(corr), flat(corr), -2.0, 2.0, ALU.max,
                              ALU.min)
            vec.tensor_tensor(flat(t1), flat(orth), flat(corr), ALU.subtract)

            t0c = CHUNKS[ci][0]
            yv = AP(y_flat.tensor, y_flat.offset + t0c * E,
                    [list(y_flat.ap[0]), [1, NE]])
            nc.sync.dma_start(out=yv, in_=flat(t1))

        # ---------------- main tile loop -----------------------------------
        cam_c = None
        chunk_of = {}
        for ci, (t0, t1_) in enumerate(CHUNKS):
            for t in range(t0, t1_):
                chunk_of[t] = (ci, t0, t1_)

        grp_of = {}
        tg = 0
        for gsz in DMA_GROUPS:
            for t in range(tg, tg + gsz):
                grp_of[t] = (tg, gsz)
            tg += gsz

        xt16 = None
        S_last = None
        for t in range(TPC):
            ci, t0, t1_ = chunk_of[t]
            last_chunk = (ci == len(CHUNKS) - 1)
            if t == t0:
                cam_c = campool.tile([P, NMAX * E], f32, tag="cam",
                                     name=f"cam{ci}")
                if last_chunk:
                    S_last = so3_begin(ci, t1_ - t0)
            g0, gsz = grp_of[t]
            if t == g0:
                xt16 = x16pool.tile([P, 2 * C * E], F16, tag="xt16",
                                    name=f"xt16_{t}")
                # SWDGE DMA with inline fp32->fp16 cast (read-bound on HBM)
                nc.gpsimd.dma_start(out=xt16[:, :gsz * C * E],
                                    in_=x_tiled[:, g0:g0 + gsz, :])
            toff = (t - g0) * C * E
            xT = x16pool.tile([P, C * E], F16, tag="xT", name=f"xT{t}")
            for g, (c0_, nch) in enumerate(((0, 8), (8, 8), (16, 2))):
                pt = tpp.tile([P, 1024], F16, tag="pt", name=f"pt{t}_{g}")
                for a in range(nch):
                    j = c0_ + a
                    nc.tensor.transpose(pt[:, P * a:P * (a + 1)],
                                        xt16[:, toff + P * j:toff + P * (j + 1)],
                                        idt[:])
                on_dve = (t in DVE_COPY_TILES) or \
                    (t >= SPLIT_COPY_TILES and g == 1)
                if on_dve:
                    # int32 reinterpret: bit-exact on DVE (ACT would round)
                    vec.tensor_copy(
                        xT[:, P * c0_:P * (c0_ + nch)].bitcast(mybir.dt.int32),
                        pt[:, :P * nch].bitcast(mybir.dt.int32))
                else:
                    act.copy(xT[:, P * c0_:P * (c0_ + nch)], pt[:, :P * nch])
            pc = pcp.tile([P, E], f32, tag="pc", name=f"pc{t}")
            for j in range(NCH):
                nc.tensor.matmul(pc[:], xT[:, P * j:P * (j + 1)],
                                 v(wm_sb, E * j, (1, E)),
                                 start=(j == 0), stop=(j == NCH - 1))
            if t >= SPLIT_COPY_TILES:
                vec.tensor_copy(v(cam_c, (t - t0) * E, (1, E)), pc[:])
            else:
                act.copy(v(cam_c, (t - t0) * E, (1, E)), pc[:])
            if last_chunk:
                # pipeline Newton iter-0 per tile during the loop
                so3_iter0_tile(S_last, cam_c, t - t0)
            if t == t1_ - 1:
                if last_chunk:
                    so3_rest(S_last, cam_c, True)
                else:
                    S = so3_begin(ci, t1_ - t0)
                    so3_rest(S, cam_c, False)


def build(b_local=B_LOCAL):
    nc = bacc.Bacc("TRN2", target_bir_lowering=False, debug=False)
    x = nc.dram_tensor("x", [b_local, C, 3, 3], F32, kind="ExternalInput")
    wm = nc.dram_tensor("wm", [P, NCH * E], F16, kind="ExternalInput")
    idt = nc.dram_tensor("idt", [P, P], F16, kind="ExternalInput")
    y = nc.dram_tensor("y", [b_local, 3, 3], F32, kind="ExternalOutput")
    with TileContext(nc) as tc:
        _emit(nc, tc, x.ap(), wm.ap(), idt.ap(), y.ap())
    nc.compile()
    return nc


_NC_CACHE = {}


def kernel(x: np.ndarray, W: np.ndarray) -> np.ndarray:
    assert x.shape == (B_FULL, C, 3, 3) and W.shape == (C,)
    if "nc" not in _NC_CACHE:
        _NC_CACHE["nc"] = build()
    nc = _NC_CACHE["nc"]
    xs = np.ascontiguousarray(x.reshape(N_CORES, B_LOCAL, C, 3, 3))
    wmn = make_wm(np.asarray(W, dtype=np.float32))
    idn = np.eye(P, dtype=np.float16)
    in_maps = [{"x": xs[i], "wm": wmn, "idt": idn} for i in range(N_CORES)]
    res = bass_utils.run_bass_kernel_spmd(nc, in_maps, core_ids=list(range(N_CORES)))
    return np.concatenate([r["y"] for r in res.results], axis=0)


if __name__ == "__main__":
    rng = np.random.default_rng(0)
    x = rng.standard_normal((B_FULL, C, 3, 3), dtype=np.float32)
    W = (rng.standard_normal(C, dtype=np.float32) / np.sqrt(C)).astype(np.float32)
    out = kernel(x=x, W=W)
    print(out.shape, out.dtype)


# revision 34
# speedup vs baseline: 1.0023x; 1.0023x over previous
"""Trainium2 Bass kernel for nn_CameraEstimator.

Computes, for each batch item b:
    camera[b] = einsum('chw,c->hw', x[b], W)          (C=256 contraction)
    out[b]    = nearest-rotation(camera[b])           (SVD u@vh + det reflection fix)

Pipeline per 128-row tile (32 tiles/core):
    SWDGE DMA with inline fp32->fp16 cast -> PE transpose (18x [128,128]) ->
    DVE/ACT copy PSUM->SBUF -> PE matmul vs masked-W (accumulate C) -> cam.
The SO(3) projection (scaled Newton polar + closed-form smallest-eigenvalue
reflection fix) runs on DVE/ACT in 3 chunks interleaved with the tile loop.
To shorten the serial dependency chain of the last chunk, the reflection's
P-factor is computed from the 2nd Newton iterate (numerically equivalent:
rel err 7.8e-4 either way) so the eigen/projector branch overlaps Newton
iterations 3-4.  All ACT transcendentals are Sqrt (one table set);
sin(acos(r)/3+pi/6) is a pair of factored quartics P(r)+sqrt(1-r)*Q(r).

Sharding: batch dim split evenly across 8 NeuronCores (data parallel), W
replicated.
"""

import numpy as np

import concourse.bacc as bacc
import concourse.bass as bass
import concourse.mybir as mybir
from concourse.bass_types import AP
from concourse.tile import TileContext
from concourse import bass_utils

F32 = mybir.dt.float32
F16 = mybir.dt.float16
ALU = mybir.AluOpType
ACT = mybir.ActivationFunctionType

B_FULL = 32768
C = 256
E = 9
N_CORES = 8
P = 128
B_LOCAL = B_FULL // N_CORES          # 4096
TPC = B_LOCAL // P                   # 32 matrices per partition
NCH = (C * E) // P                   # 18 chunks of 128 per tile

CHUNKS = [(0, 16), (16, 26), (26, 32)]
DVE_COPY_TILES = {1, 4, 7, 10, 13, 16, 19, 22, 25}  # PSUM copies on DVE
SPLIT_COPY_TILES = 28                 # tiles >= this split copies DVE/ACT
# x DMA grouping: small transfers first (fast pipeline start), then 4-tile
# groups (fewer transfers -> no completion-sem reuse throttling)
DMA_GROUPS = [1] * TPC

# sin(acos(r)/3 + pi/6) ~= c4P*(r^2+pb1*r+pb0)(r^2+pb3*r+pb2)
#                        + sqrt(1-r)*c4Q*(...)
C4P = -0.00012669774781398082
PB = [(-10.373862952604547, -41.02339770448056),
      (-6.311076547771094, 83.61692215433715)]
C4Q = 1.3807082105156778e-05
QB = [(-46.779313658342474, 452.0278334014067),
      (2.6583141961744774, 69.12475662024718)]


def v(base: AP, off: int, *dims) -> AP:
    """Free-dim view of an SBUF tile AP: keep partition dim, set free dims."""
    return AP(base.tensor, base.offset + off,
              [list(base.ap[0])] + [[s, c] for (s, c) in dims])


def make_wm(W: np.ndarray) -> np.ndarray:
    """Masked-W moving operand, partition-major: wm[k, j*9+m] = fp16(W[c])
    where c=(128j+k)//9 if (128j+k)%9 == m else 0, so that
    xT16_j.T @ wm[:, 9j:9j+9] accumulates camera.  Layout [P, NCH*E] keeps
    the DMA contiguous per partition (tiny-descriptor DMAs stall the SDMA
    round-robin and starve the x stream)."""
    kidx = np.arange(C * E)
    wh = np.zeros((C * E, E), np.float32)
    wh[kidx, kidx % E] = W[kidx // E]
    wjkm = wh.astype(np.float16).reshape(NCH, P, E)
    return np.ascontiguousarray(wjkm.transpose(1, 0, 2).reshape(P, NCH * E))


def _emit(nc, tc, x_ap, wm_ap, idt_ap, y_ap):
    f32 = F32
    vec = nc.vector
    act = nc.scalar

    x_flat = x_ap.rearrange("b c h w -> b (c h w)")
    x_tiled = x_flat.rearrange("(p t) f -> p t f", p=P)
    y_flat = y_ap.rearrange("b h w -> b (h w)").rearrange("(p t) e -> p (t e)", p=P)

    NMAX = max(t1 - t0 for t0, t1 in CHUNKS)

    with tc.tile_pool(name="x16", bufs=8) as x16pool, \
         tc.tile_pool(name="tp", bufs=6, space="PSUM") as tpp, \
         tc.tile_pool(name="pcp", bufs=2, space="PSUM") as pcp, \
         tc.tile_pool(name="cam", bufs=2) as campool, \
         tc.tile_pool(name="ck", bufs=3) as ck, \
         tc.tile_pool(name="wk", bufs=1) as wp:

        idt = wp.tile([P, P], F16)
        nc.sync.dma_start(out=idt[:], in_=idt_ap)
        wm_sb = wp.tile([P, NCH * E], F16)
        nc.sync.dma_start(out=wm_sb[:], in_=wm_ap)

        _consts = {}

        def cb(val):
            if val not in _consts:
                ct = wp.tile([P, 1], f32, name=f"const{len(_consts)}")
                vec.memset(ct[:], float(val))
                _consts[val] = ct[:]
            return _consts[val]

        # ---------------- SO(3) projection for one chunk -------------------
        # mu schedule per Newton iter: "38" = |det|^(-3/8) (4 sqrt levels),
        # "12" = |det|^(-1/2) (2 sqrt levels), None = unscaled.
        MU_SCHED = ("38", "38", "12", None)

        def so3_begin(ci, n):
            nm = f"c{ci}"

            def big(tag):
                return ck.tile([P, NMAX * E], f32, tag=tag, name=f"{tag}{nm}")

            def plane(tag):
                return ck.tile([P, NMAX], f32, tag=tag, name=f"{tag}{nm}")

            S = {"ci": ci, "n": n, "last": ci >= 1}
            for t_ in ("Ya", "Yb", "Yc", "CfN", "t1", "t2",
                       "CfR", "ra", "rb", "rc"):
                S[t_] = big(t_)
            S["D"] = ck.tile([P, NMAX * 36], f32, tag="D", name=f"D{nm}")
            S["D2"] = ck.tile([P, NMAX * 36], f32, tag="D2", name=f"D2{nm}")
            S["td"] = ck.tile([P, NMAX * 3], f32, tag="td", name=f"td{nm}")
            S["td2"] = ck.tile([P, NMAX * 3], f32, tag="td2", name=f"td2{nm}")
            for t_ in ("det", "det0", "rdet", "mi", "u1", "u2", "u3",
                       "c2", "c1", "c0", "q", "r", "p26", "pp",
                       "sq", "ha", "hb", "hp", "s3", "w1", "plv"):
                S[t_] = plane(t_)
            return S

        def matv(tile, k0, m, off=0):
            return v(tile, k0 * E + off, (E, m), (3, 3), (1, 3))

        def flatv(tile, k0, m):
            return v(tile, k0 * E, (1, m * E))

        def row0v(tile, k0, m):
            return v(tile, k0 * E, (E, m), (1, 3))

        def diagv(tile, k0, m):
            return v(tile, k0 * E, (E, m), (4, 3))

        def plv_(tile, k0, m):
            return v(tile, k0, (1, m))

        def bc9v(tile, k0, m):
            return v(tile, k0, (1, m), (0, E))

        def bc3v(tile, k0, m):
            return v(tile, k0, (1, m), (0, 3))

        def build_D(S, Y, Dst, k0, m):
            src = v(Y, k0 * E, (E, m), (3, 3), (0, 2), (1, 3))
            eng = vec.tensor_copy if S["last"] else act.copy
            for off in (0, 18):
                eng(v(Dst, k0 * 36 + off, (36, m), (6, 3), (3, 2), (1, 3)),
                    src)

        def cofactor(S, Y, out, Dst, ta, tb, k0, m):
            def db(off):
                return v(Dst, k0 * 36 + off, (36, m), (6, 3), (1, 3))
            build_D(S, Y, Dst, k0, m)
            vec.tensor_tensor(matv(ta, k0, m), db(7), db(14), ALU.mult)
            vec.tensor_tensor(matv(tb, k0, m), db(8), db(13), ALU.mult)
            vec.tensor_tensor(matv(out, k0, m), matv(ta, k0, m),
                              matv(tb, k0, m), ALU.subtract)

        def det_of(S, Y, Cof, out, tdx, k0, m):
            vec.tensor_tensor(v(tdx, k0 * 3, (3, m), (1, 3)),
                              row0v(Y, k0, m), row0v(Cof, k0, m), ALU.mult)
            vec.tensor_reduce(plv_(out, k0, m), v(tdx, k0 * 3, (3, m), (1, 3)),
                              mybir.AxisListType.X, ALU.add)

        def newton_iter(S, Y, Yn, it, mu, k0, m):
            det, rdet, mi = S["det"], S["rdet"], S["mi"]
            u1, u2 = S["u1"], S["u2"]
            cofactor(S, Y, S["CfN"], S["D"], S["t1"], S["t2"], k0, m)
            det_of(S, Y, S["CfN"], det, S["td"], k0, m)
            if it == 0:
                vec.tensor_copy(plv_(S["det0"], k0, m), plv_(det, k0, m))
            vec.reciprocal(plv_(rdet, k0, m), plv_(det, k0, m))
            if mu == "38":
                # 1/mu = n^(1/4)*n^(1/8), n = sqrt(det^2)
                vec.tensor_tensor(plv_(u1, k0, m), plv_(det, k0, m),
                                  plv_(det, k0, m), ALU.mult)
                act.activation(plv_(u1, k0, m), plv_(u1, k0, m), ACT.Sqrt,
                               bias=cb(1e-35))
                act.activation(plv_(u1, k0, m), plv_(u1, k0, m), ACT.Sqrt,
                               bias=cb(0.0))
                act.activation(plv_(u2, k0, m), plv_(u1, k0, m), ACT.Sqrt,
                               bias=cb(0.0))
                act.activation(plv_(S["u3"], k0, m), plv_(u2, k0, m), ACT.Sqrt,
                               bias=cb(0.0))
                vec.tensor_tensor(plv_(mi, k0, m), plv_(u2, k0, m),
                                  plv_(S["u3"], k0, m), ALU.mult)
            elif mu == "12":
                # 1/mu = n^(1/2)
                vec.tensor_tensor(plv_(u1, k0, m), plv_(det, k0, m),
                                  plv_(det, k0, m), ALU.mult)
                act.activation(plv_(u1, k0, m), plv_(u1, k0, m), ACT.Sqrt,
                               bias=cb(1e-35))
                act.activation(plv_(mi, k0, m), plv_(u1, k0, m), ACT.Sqrt,
                               bias=cb(0.0))
            if mu is not None:
                vec.scalar_tensor_tensor(plv_(u2, k0, m), plv_(mi, k0, m), 0.5,
                                         plv_(rdet, k0, m), ALU.mult, ALU.mult)
                vec.reciprocal(plv_(u1, k0, m), plv_(mi, k0, m))
                vec.tensor_tensor(flatv(S["t1"], k0, m), flatv(Y, k0, m),
                                  bc9v(u1, k0, m), ALU.mult)
                vec.tensor_tensor(flatv(S["t2"], k0, m), flatv(S["CfN"], k0, m),
                                  bc9v(u2, k0, m), ALU.mult)
                vec.scalar_tensor_tensor(flatv(Yn, k0, m), flatv(S["t1"], k0, m),
                                         0.5, flatv(S["t2"], k0, m),
                                         ALU.mult, ALU.add)
            else:
                vec.scalar_tensor_tensor(flatv(S["t2"], k0, m),
                                         flatv(S["CfN"], k0, m), 0.5,
                                         bc9v(rdet, k0, m), ALU.mult, ALU.mult)
                vec.scalar_tensor_tensor(flatv(Yn, k0, m), flatv(Y, k0, m),
                                         0.5, flatv(S["t2"], k0, m),
                                         ALU.mult, ALU.add)

        def so3_iter0(S, cam_c, k0, m):
            newton_iter(S, cam_c, S["Ya"], 0, MU_SCHED[0], k0, m)

        def so3_rest(S, cam_c, pre0):
            ci, n = S["ci"], S["n"]
            NE = n * E

            def mat(t_, off=0):
                return matv(t_, 0, n, off)

            def flat(t_):
                return flatv(t_, 0, n)

            def diag(t_):
                return diagv(t_, 0, n)

            def pl(t_):
                return plv_(t_, 0, n)

            def bc9(t_):
                return bc9v(t_, 0, n)

            def bc3(t_):
                return bc3v(t_, 0, n)

            Ya, Yb, Yc = S["Ya"], S["Yb"], S["Yc"]
            CfR, ra, rb, rc = S["CfR"], S["ra"], S["rb"], S["rc"]
            t1 = S["t1"]
            det0 = S["det0"]
            c2, c1, c0 = S["c2"], S["c1"], S["c0"]
            q, r, p26, pp = S["q"], S["r"], S["p26"], S["pp"]
            sq, ha, hb, hp = S["sq"], S["ha"], S["hb"], S["hp"]
            s3, w1, plv = S["s3"], S["w1"], S["plv"]

            if not pre0:
                newton_iter(S, cam_c, Ya, 0, MU_SCHED[0], 0, n)
            newton_iter(S, Ya, Yb, 1, MU_SCHED[1], 0, n)

            # ---- reflection prep from Y2 (=Yb), concurrent with iters 3-4 --
            Pm = rb
            for k in range(3):
                a = v(Yb, 3 * k, (E, n), (1, 3), (0, 3))
                b = v(cam_c, 3 * k, (E, n), (0, 3), (1, 3))
                dst = (Pm, ra, CfR)[k]
                vec.tensor_tensor(mat(dst), a, b, ALU.mult)
            vec.tensor_tensor(mat(ra), mat(ra), mat(CfR), ALU.add)
            vec.tensor_tensor(mat(Pm), mat(Pm), mat(ra), ALU.add)

            cofactor(S, Pm, CfR, S["D2"], ra, rc, 0, n)
            vec.tensor_reduce(pl(c2), diag(Pm), mybir.AxisListType.X, ALU.add)
            vec.tensor_reduce(pl(c1), diag(CfR), mybir.AxisListType.X, ALU.add)
            det_of(S, Pm, CfR, c0, S["td2"], 0, n)

            vec.tensor_scalar(pl(q), pl(c2), 1.0 / 3.0, None, ALU.mult)
            vec.scalar_tensor_tensor(pl(p26), pl(c2), 1.0 / 9.0, pl(c2),
                                     ALU.mult, ALU.mult)
            vec.scalar_tensor_tensor(pl(p26), pl(c1), -1.0 / 3.0, pl(p26),
                                     ALU.mult, ALU.add)
            vec.tensor_scalar(pl(p26), pl(p26), 0.0, None, ALU.max)
            act.activation(pl(pp), pl(p26), ACT.Sqrt, bias=cb(1e-30))
            vec.scalar_tensor_tensor(pl(r), pl(c2), 2.0 / 9.0, pl(c2),
                                     ALU.mult, ALU.mult)
            vec.tensor_tensor(pl(r), pl(r), pl(c1), ALU.subtract)
            vec.tensor_tensor(pl(r), pl(r), pl(q), ALU.mult)
            vec.tensor_tensor(pl(r), pl(r), pl(c0), ALU.add)
            vec.scalar_tensor_tensor(pl(plv), pl(p26), 2.0, pl(pp),
                                     ALU.mult, ALU.mult)
            vec.tensor_scalar(pl(plv), pl(plv), 1e-30, None, ALU.add)
            vec.reciprocal(pl(plv), pl(plv))
            vec.tensor_tensor(pl(r), pl(r), pl(plv), ALU.mult)
            vec.tensor_scalar(pl(r), pl(r), -1.0, 1.0, ALU.max, ALU.min)
            act.activation(pl(sq), pl(r), ACT.Sqrt, scale=-1.0, bias=cb(1.0))
            vec.tensor_tensor(pl(S["u3"]), pl(r), pl(r), ALU.mult)  # r^2
            r2 = S["u3"]
            vec.scalar_tensor_tensor(pl(ha), pl(r), PB[0][0], pl(r2),
                                     ALU.mult, ALU.add)
            vec.tensor_scalar(pl(ha), pl(ha), PB[0][1], None, ALU.add)
            vec.scalar_tensor_tensor(pl(hb), pl(r), PB[1][0], pl(r2),
                                     ALU.mult, ALU.add)
            vec.tensor_scalar(pl(hb), pl(hb), PB[1][1], None, ALU.add)
            vec.scalar_tensor_tensor(pl(hp), pl(ha), C4P, pl(hb),
                                     ALU.mult, ALU.mult)
            vec.scalar_tensor_tensor(pl(ha), pl(r), QB[0][0], pl(r2),
                                     ALU.mult, ALU.add)
            vec.tensor_scalar(pl(ha), pl(ha), QB[0][1], None, ALU.add)
            vec.scalar_tensor_tensor(pl(hb), pl(r), QB[1][0], pl(r2),
                                     ALU.mult, ALU.add)
            vec.tensor_scalar(pl(hb), pl(hb), QB[1][1], None, ALU.add)
            vec.scalar_tensor_tensor(pl(ha), pl(ha), C4Q, pl(hb),
                                     ALU.mult, ALU.mult)
            vec.tensor_tensor(pl(ha), pl(ha), pl(sq), ALU.mult)
            vec.tensor_tensor(pl(hp), pl(hp), pl(ha), ALU.add)
            vec.scalar_tensor_tensor(pl(s3), pl(pp), -2.0, pl(hp),
                                     ALU.mult, ALU.mult)
            vec.tensor_tensor(pl(s3), pl(s3), pl(q), ALU.add)
            # Nadj = CP + s3*P + (s3^2 - s3*c2) I; proj = Nadj/tr * mask
            vec.scalar_tensor_tensor(pl(w1), pl(c2), -1.0, pl(s3),
                                     ALU.mult, ALU.add)
            vec.tensor_tensor(pl(w1), pl(w1), pl(s3), ALU.mult)
            vec.tensor_tensor(flat(ra), flat(Pm), bc9(s3), ALU.mult)
            vec.tensor_tensor(flat(CfR), flat(CfR), flat(ra), ALU.add)
            vec.tensor_tensor(diag(CfR), diag(CfR), bc3(w1), ALU.add)
            vec.tensor_reduce(pl(plv), diag(CfR), mybir.AxisListType.X, ALU.add)
            vec.tensor_scalar(pl(plv), pl(plv), 1e-30, None, ALU.add)
            vec.reciprocal(pl(plv), pl(plv))
            # fold the det0<0 reflection mask into proj (off the tail path)
            vec.tensor_scalar(pl(ha), pl(det0), 0.0, 2.0, ALU.is_lt, ALU.mult)
            vec.tensor_tensor(pl(plv), pl(plv), pl(ha), ALU.mult)
            vec.tensor_tensor(flat(CfR), flat(CfR), bc9(plv), ALU.mult)

            # Newton iters 3-4 (emitted after prep; scheduler overlaps)
            newton_iter(S, Yb, Yc, 2, MU_SCHED[2], 0, n)
            newton_iter(S, Yc, Ya, 3, MU_SCHED[3], 0, n)
            orth = Ya

            # corr = orth @ proj (tree); R = orth - clamp(corr)
            corr = rb  # Pm dead after Nadj
            for k in range(3):
                a = v(orth, k, (E, n), (3, 3), (0, 3))
                b = v(CfR, 3 * k, (E, n), (0, 3), (1, 3))
                dst = (corr, ra, t1)[k]
                vec.tensor_tensor(mat(dst), a, b, ALU.mult)
            vec.tensor_tensor(mat(ra), mat(ra), mat(t1), ALU.add)
            vec.tensor_tensor(mat(corr), mat(corr), mat(ra), ALU.add)
            vec.tensor_scalar(flat(corr), flat(corr), -2.0, 2.0, ALU.max,
                              ALU.min)
            vec.tensor_tensor(flat(t1), flat(orth), flat(corr), ALU.subtract)

            t0c = CHUNKS[ci][0]
            yv = AP(y_flat.tensor, y_flat.offset + t0c * E,
                    [list(y_flat.ap[0]), [1, NE]])
            nc.sync.dma_start(out=yv, in_=flat(t1))

        # ---------------- main tile loop -----------------------------------
        cam_c = None
        chunk_of = {}
        for ci, (t0, t1_) in enumerate(CHUNKS):
            for t in range(t0, t1_):
                chunk_of[t] = (ci, t0, t1_)

        grp_of = {}
        tg = 0
        for gsz in DMA_GROUPS:
            for t in range(tg, tg + gsz):
                grp_of[t] = (tg, gsz)
            tg += gsz

        xt16 = None
        S_map = {}
        for t in range(TPC):
            ci, t0, t1_ = chunk_of[t]
            pipelined = (ci >= 1)   # chunks 1+ run Newton iter-0 per tile-pair
            if t == t0:
                cam_c = campool.tile([P, NMAX * E], f32, tag="cam",
                                     name=f"cam{ci}")
                if pipelined:
                    S_map[ci] = so3_begin(ci, t1_ - t0)
            g0, gsz = grp_of[t]
            if t == g0:
                xt16 = x16pool.tile([P, 2 * C * E], F16, tag="xt16",
                                    name=f"xt16_{t}")
                # SWDGE DMA with inline fp32->fp16 cast (read-bound on HBM)
                nc.gpsimd.dma_start(out=xt16[:, :gsz * C * E],
                                    in_=x_tiled[:, g0:g0 + gsz, :])
            toff = (t - g0) * C * E
            xT = x16pool.tile([P, C * E], F16, tag="xT", name=f"xT{t}")
            for g, (c0_, nch) in enumerate(((0, 8), (8, 8), (16, 2))):
                pt = tpp.tile([P, 1024], F16, tag="pt", name=f"pt{t}_{g}")
                for a in range(nch):
                    j = c0_ + a
                    nc.tensor.transpose(pt[:, P * a:P * (a + 1)],
                                        xt16[:, toff + P * j:toff + P * (j + 1)],
                                        idt[:])
                on_dve = (t in DVE_COPY_TILES) or \
                    (t >= SPLIT_COPY_TILES and g == 1)
                if on_dve:
                    # int32 reinterpret: bit-exact on DVE (ACT would round)
                    vec.tensor_copy(
                        xT[:, P * c0_:P * (c0_ + nch)].bitcast(mybir.dt.int32),
                        pt[:, :P * nch].bitcast(mybir.dt.int32))
                else:
                    act.copy(xT[:, P * c0_:P * (c0_ + nch)], pt[:, :P * nch])
            pc = pcp.tile([P, E], f32, tag="pc", name=f"pc{t}")
            for j in range(NCH):
                nc.tensor.matmul(pc[:], xT[:, P * j:P * (j + 1)],
                                 v(wm_sb, E * j, (1, E)),
                                 start=(j == 0), stop=(j == NCH - 1))
            if t >= SPLIT_COPY_TILES:
                vec.tensor_copy(v(cam_c, (t - t0) * E, (1, E)), pc[:])
            else:
                act.copy(v(cam_c, (t - t0) * E, (1, E)), pc[:])
            if pipelined and (t - t0) % 2 == 1:
                # pipeline Newton iter-0 per tile-pair during the loop
                so3_iter0(S_map[ci], cam_c, t - t0 - 1, 2)
            if t == t1_ - 1:
                if pipelined:
                    so3_rest(S_map[ci], cam_c, True)
                else:
                    S = so3_begin(ci, t1_ - t0)
                    so3_rest(S, cam_c, False)


def build(b_local=B_LOCAL):
    nc = bacc.Bacc("TRN2", target_bir_lowering=False, debug=False)
    x = nc.dram_tensor("x", [b_local, C, 3, 3], F32, kind="ExternalInput")
    wm = nc.dram_tensor("wm", [P, NCH * E], F16, kind="ExternalInput")
    idt = nc.dram_tensor("idt", [P, P], F16, kind="ExternalInput")
    y = nc.dram_tensor("y", [b_local, 3, 3], F32, kind="ExternalOutput")
    with TileContext(nc) as tc:
        _emit(nc, tc, x.ap(), wm.ap(), idt.ap(), y.ap())
    nc.compile()
    return nc


_NC_CACHE = {}


def kernel(x: np.ndarray, W: np.ndarray) -> np.ndarray:
    assert x.shape == (B_FULL, C, 3, 3) and W.shape == (C,)
    if "nc" not in _NC_CACHE:
        _NC_CACHE["nc"] = build()
    nc = _NC_CACHE["nc"]
    xs = np.ascontiguousarray(x.reshape(N_CORES, B_LOCAL, C, 3, 3))
    wmn = make_wm(np.asarray(W, dtype=np.float32))
    idn = np.eye(P, dtype=np.float16)
    in_maps = [{"x": xs[i], "wm": wmn, "idt": idn} for i in range(N_CORES)]
    res = bass_utils.run_bass_kernel_spmd(nc, in_maps, core_ids=list(range(N_CORES)))
    return np.concatenate([r["y"] for r in res.results], axis=0)


if __name__ == "__main__":
    rng = np.random.default_rng(0)
    x = rng.standard_normal((B_FULL, C, 3, 3), dtype=np.float32)
    W = (rng.standard_normal(C, dtype=np.float32) / np.sqrt(C)).astype(np.float32)
    out = kernel(x=x, W=W)
    print(out.shape, out.dtype)
